# revision 1
# baseline (speedup 1.0000x reference)
"""Trainium2 Bass kernel for nn_DecoderRNN (LSTM decoder with argmax feedback).

Structure: V-sharded tensor-parallel over 8 cores.
- out-matmul (h @ W_fc.T): each core computes its 1024-wide V shard,
  2x column-tiled on the PE (batch 64 -> both column halves of the array).
- x @ W_ih.T is computed incrementally: x is a cumulative sum of one-hots,
  so each step gathers one row of W_ih.T (per batch element) via indirect
  DMA and accumulates into the gate pre-activations.
- LSTM cell H-sharded (each core owns a 128-wide H slice); h chunks are
  all-gathered each step via XOR-relative remote SBUF-to-SBUF DMA
  broadcasts (one-hot dests), as is the per-core argmax candidate set.
- sigmoid computed as 0.5*(1+tanh(x/2)) (tanh table is 4-ULP; sigmoid's is
  40-ULP). The factor 2 in h'=2h is folded into W_fc/2 on the host.
- argmax tie-break matches jnp.argmax (first occurrence) via
  code = 8191 - global_idx and max-reductions.
"""
import numpy as np

V, H, B, NC = 8192, 1024, 64, 8
HSH = H // NC    # 128
VSH = V // NC    # 1024


PHYS = [0, 1, 2, 3, 6, 7, 4, 5]


def _dphys(d):
    # logical->physical NC map on trn2 is [0,1,2,3,6,7,4,5]; XOR-deltas map to
    return d ^ ((d & 4) >> 1)


def build_program(T):
    import concourse.mybir as mybir
    from concourse import bass, bacc, tile
    from concourse.bass import _add_dep_helper as dep

    fp32 = mybir.dt.float32
    AF = mybir.ActivationFunctionType
    OP = mybir.AluOpType

    nc = bacc.Bacc("TRN2", target_bir_lowering=False, debug=False,
                   num_devices=NC, num_swdge_queues=4)

    # ---- I/O ----
    d_wih = nc.dram_tensor("wih", [V, 4 * HSH], fp32, kind="ExternalInput").ap()
    d_wfc = nc.dram_tensor("wfc", [NC, HSH, VSH], fp32, kind="ExternalInput").ap()
    d_bias = nc.dram_tensor("bias", [1, VSH], fp32, kind="ExternalInput").ap()
    d_hh = nc.dram_tensor("hh", [B, 4 * HSH], fp32, kind="ExternalInput").ap()
    d_cb = nc.dram_tensor("cb", [128, 1], fp32, kind="ExternalInput").ap()
    d_id = nc.dram_tensor("ident", [B, B], fp32, kind="ExternalInput").ap()
    d_pm = nc.dram_tensor("perm", [128, B], fp32, kind="ExternalInput").ap()
    d_out = nc.dram_tensor("out", [T, B, VSH], fp32, kind="ExternalOutput").ap()

    with tile.TileContext(nc) as tc:
        # ---- persistent SBUF ----
        wfc = nc.alloc_sbuf_tensor("wfc_sb", [128, NC * VSH], fp32).ap()
        bias = nc.alloc_sbuf_tensor("bias_sb", [1, VSH], fp32).ap()
        gacc = nc.alloc_sbuf_tensor("gacc_sb", [B, 4 * HSH], fp32).ap()
        cC = nc.alloc_sbuf_tensor("c_sb", [B, HSH], fp32).ap()
        hts = [nc.alloc_sbuf_tensor(f"hts{i}", [128, B], fp32).ap() for i in range(2)]
        htb = [nc.alloc_sbuf_tensor(f"htb{i}", [128, 7 * B], fp32).ap() for i in range(2)]
        mvg = [nc.alloc_sbuf_tensor(f"mvg{i}", [128, 2], fp32).ap() for i in range(2)]
        rvg = [nc.alloc_sbuf_tensor(f"rvg{i}", [128, 16], fp32).ap() for i in range(2)]
        cb = nc.alloc_sbuf_tensor("cb_sb", [128, 1], fp32).ap()
        idn = nc.alloc_sbuf_tensor("idn_sb", [B, B], fp32).ap()
        perm = nc.alloc_sbuf_tensor("perm_sb", [128, B], fp32).ap()
        ones1 = nc.alloc_sbuf_tensor("ones1_sb", [1, B], fp32).ap()
        idx_sb = nc.alloc_sbuf_tensor("idx_sb", [B, 1], mybir.dt.int32).ap()
        pmin = nc.alloc_sbuf_tensor("pmin_sb", [128, 2], fp32).ap()

        a_ht = nc.alloc_semaphore("a_ht")
        a_vg = nc.alloc_semaphore("a_vg")
        s_ht = [nc.alloc_semaphore(f"s_ht{q}") for q in range(3)]
        s_vg = [nc.alloc_semaphore(f"s_vg{q}") for q in range(3)]
        QINC = (48, 32, 32)  # per-queue local-sem inc per step (3,2,2 bcasts)

        # ---- init ----
        for s in range(NC):
            nc.sync.dma_start(wfc[:, s * VSH:(s + 1) * VSH], d_wfc[s])
        nc.sync.dma_start(bias[:], d_bias[:])
        nc.sync.dma_start(gacc[:], d_hh[:])
        nc.sync.dma_start(cb[:], d_cb[:])
        nc.sync.dma_start(idn[:], d_id[:])
        nc.sync.dma_start(perm[:], d_pm[:])
        nc.vector.memset(cC[:], 0.0)
        nc.vector.memset(ones1[:], 1.0)

        ENG = {1: 0, 2: 1, 3: 2, 4: 4, 5: 5, 6: 6, 7: 7}  # same-die d<4 -> eng 0-3; D2D d>=4 -> eng 4-7
        last_pool = None  # chains exchange blocks in Pool-stream order

        def bcast(src, dst_of, rsem, lsem):
            """7 one-hot remote broadcasts spread over queues 1..3."""
            nonlocal last_pool
            first = None
            for d in range(1, 8):
                rd = [None] * 8
                rd[d] = (0, _dphys(d))
                ins = nc.gpsimd.remote_dma_broadcast(
                    out_ap=dst_of(d), in_ap=src[:],
                    remote_sem=rsem, local_sem=lsem[(d - 1) % 3], rdests=rd,
                    queue_num=1 + (d - 1) % 3)
                if first is None:
                    first = ins
            trig = None
            for q in (1, 2, 3):
                trig = nc.gpsimd.trigger_dma(count=None, queue_num=q)
            if last_pool is not None:
                dep(first.ins, last_pool.ins, reason="ring order")
            last_pool = trig
            return trig

        with tc.tile_pool(name="sb", bufs=3) as sb, \
             tc.tile_pool(name="sb2", bufs=2) as sb2, \
             tc.tile_pool(name="ps_out", bufs=2, space="PSUM") as ps_out, \
             tc.tile_pool(name="ps_tr", bufs=2, space="PSUM") as ps_tr, \
             tc.tile_pool(name="ps_pm", bufs=2, space="PSUM") as ps_pm, \
             tc.tile_pool(name="ps_junk", bufs=1, space="PSUM") as ps_junk:

            junk_ps = ps_junk.tile([128, 512], fp32, tag="junk")

            for t in range(T):
                bb = t % 2

                # ======== LSTM cell (gates -> c, h2=2h) ========
                tio = sb2.tile([B, 3 * HSH], fp32, tag="tio")
                tg_ = sb2.tile([B, HSH], fp32, tag="tg")
                nc.scalar.activation(tio[:], gacc[:, 0:3 * HSH], AF.Tanh, scale=0.5)
                nc.scalar.activation(tg_[:], gacc[:, 3 * HSH:4 * HSH], AF.Tanh)
                ti = tio[:, 0:HSH]
                tf = tio[:, HSH:2 * HSH]
                to = tio[:, 2 * HSH:3 * HSH]
                m1 = sb2.tile([B, HSH], fp32, tag="m1")
                s1 = sb2.tile([B, HSH], fp32, tag="s1")
                m2 = sb2.tile([B, HSH], fp32, tag="m2")
                s2 = sb2.tile([B, HSH], fp32, tag="s2")
                ss = sb2.tile([B, HSH], fp32, tag="ss")
                nc.vector.tensor_tensor(out=m1[:], in0=tf, in1=cC[:], op=OP.mult)
                nc.vector.tensor_tensor(out=s1[:], in0=cC[:], in1=m1[:], op=OP.add)
                nc.vector.tensor_tensor(out=m2[:], in0=ti, in1=tg_[:], op=OP.mult)
                nc.vector.tensor_tensor(out=s2[:], in0=tg_[:], in1=m2[:], op=OP.add)
                nc.vector.tensor_tensor(out=ss[:], in0=s1[:], in1=s2[:], op=OP.add)
                nc.vector.tensor_scalar(out=cC[:], in0=ss[:], scalar1=0.5,
                                        scalar2=None, op0=OP.mult)
                tc2 = sb2.tile([B, HSH], fp32, tag="tc2")
                nc.scalar.activation(tc2[:], ss[:], AF.Tanh, scale=0.5)
                m3 = sb2.tile([B, HSH], fp32, tag="m3")
                h2 = sb2.tile([B, HSH], fp32, tag="h2")
                nc.vector.tensor_tensor(out=m3[:], in0=to, in1=tc2[:], op=OP.mult)
                h2i = nc.vector.tensor_tensor(out=h2[:], in0=tc2[:], in1=m3[:],
                                              op=OP.add)

                # ======== transpose h2 -> hts[bb], broadcast ========
                trp = ps_tr.tile([HSH, B], fp32, tag="trp")
                trpi = nc.tensor.transpose(out=trp[:], in_=h2[:], identity=idn[:])
                with tc.tile_critical():
                    if t >= 1:
                        for q in range(3):
                            nc.vector.wait_ge(s_ht[q], QINC[q] * t)
                    hci = nc.vector.tensor_copy(out=hts[bb][:], in_=trp[:])
                bcast(hts[bb], lambda d: htb[bb][:, (d - 1) * B:d * B], a_ht, s_ht)

                # ======== out matmul: 8 K-chunks + bias row ========
                outp = ps_out.tile([128, 512], fp32, tag="outp")
                with tc.tile_critical():
                    nc.tensor.wait_ge(a_ht, 14 * (t + 1))
                    for s in range(NC):
                        lhs = hts[bb][:] if s == 0 else htb[bb][:, (s - 1) * B:s * B]
                        nc.tensor.matmul(out=outp[0:64, :], lhsT=lhs,
                                         rhs=wfc[:, s * VSH:s * VSH + 512],
                                         start=(s == 0), stop=False,
                                         tile_position=(0, 0),
                                         skip_group_check=True)
                        nc.tensor.matmul(out=outp[64:128, :], lhsT=lhs,
                                         rhs=wfc[:, s * VSH + 512:(s + 1) * VSH],
                                         start=(s == 0), stop=False,
                                         tile_position=(0, 64),
                                         skip_group_check=True)
                    nc.tensor.matmul(out=outp[0:64, :], lhsT=ones1[:],
                                     rhs=bias[:, 0:512], start=False, stop=True,
                                     tile_position=(0, 0), skip_group_check=True)
                    nc.tensor.matmul(out=outp[64:128, :], lhsT=ones1[:],
                                     rhs=bias[:, 512:1024], start=False, stop=True,
                                     tile_position=(0, 64), skip_group_check=True)

                # ======== store out (off critical path) ========
                osb = sb.tile([128, 512], fp32, tag="osb")
                nc.scalar.copy(osb[:], outp[:])
                nc.sync.dma_start(d_out[t, :, 0:512], osb[0:64, :])
                nc.sync.dma_start(d_out[t, :, 512:1024], osb[64:128, :])

                # ======== local argmax ========
                mx = sb2.tile([128, 8], fp32, tag="mx")
                mi = sb2.tile([128, 8], mybir.dt.uint32, tag="mi")
                nc.vector.max(out=mx[:], in_=outp[:])
                mii = nc.vector.max_index(out=mi[:], in_max=mx[:], in_values=outp[:])
                with tc.tile_critical():
                    if t >= 1:
                        for q in range(3):
                            nc.vector.wait_ge(s_vg[q], QINC[q] * t)
                    nc.vector.tensor_copy(out=mvg[bb][:, 0:1], in_=mx[:, 0:1])
                    # code = cb - idx   (cb = 8191 - core*1024 - half*512)
                    tsi = nc.vector.tensor_scalar(out=mvg[bb][:, 1:2], in0=mi[:, 0:1],
                                                  scalar1=-1.0, scalar2=cb[:],
                                                  op0=OP.mult, op1=OP.add)
                bcast(mvg[bb], lambda d: rvg[bb][:, 2 * d:2 * d + 2], a_vg, s_vg)

                # ======== global argmax combine ========
                sc0 = nc.vector.tensor_copy(out=rvg[bb][:, 0:2], in_=mvg[bb][:])
                vals = rvg[bb][:, 0:16:2]
                codes = rvg[bb][:, 1:16:2]
                with tc.tile_critical():
                    nc.vector.wait_ge(a_vg, 14 * (t + 1))
                    r1 = nc.vector.tensor_reduce(out=pmin[:, 0:1], in_=vals,
                                                 axis=mybir.AxisListType.X, op=OP.max)
                msk = sb2.tile([128, 8], fp32, tag="msk")
                cm = sb2.tile([128, 8], fp32, tag="cm")
                nc.vector.tensor_scalar(out=msk[:], in0=vals,
                                        scalar1=pmin[:, 0:1], scalar2=None,
                                        op0=OP.is_equal)
                nc.vector.tensor_tensor(out=cm[:], in0=msk[:], in1=codes,
                                        op=OP.mult)
                nc.vector.tensor_reduce(out=pmin[:, 1:2], in_=cm[:],
                                        axis=mybir.AxisListType.X, op=OP.max)
                pmp = ps_pm.tile([B, 2], fp32, tag="pmp")
                nc.tensor.matmul(out=pmp[:], lhsT=perm[:], rhs=pmin[:],
                                 start=True, stop=True)
                geh = sb2.tile([B, 1], fp32, tag="geh")
                gel = sb2.tile([B, 1], fp32, tag="gel")
                ch = sb2.tile([B, 1], fp32, tag="ch")
                cl = sb2.tile([B, 1], fp32, tag="cl")
                cw = sb2.tile([B, 1], fp32, tag="cw")
                nc.vector.tensor_tensor(out=geh[:], in0=pmp[:, 0:1],
                                        in1=pmin[0:64, 0:1], op=OP.is_ge)
                nc.vector.tensor_tensor(out=gel[:], in0=pmin[0:64, 0:1],
                                        in1=pmp[:, 0:1], op=OP.is_ge)
                nc.vector.tensor_tensor(out=ch[:], in0=pmp[:, 1:2], in1=geh[:],
                                        op=OP.mult)
                nc.vector.tensor_tensor(out=cl[:], in0=pmin[0:64, 1:2], in1=gel[:],
                                        op=OP.mult)
                nc.vector.tensor_tensor(out=cw[:], in0=ch[:], in1=cl[:], op=OP.max)
                nc.vector.tensor_scalar(out=idx_sb[:], in0=cw[:], scalar1=-1.0,
                                        scalar2=8191.0, op0=OP.mult, op1=OP.add)

                # ======== gather W_ih.T rows, update gate accumulator ========
                if t < T - 1:
                    gat = sb.tile([B, 4 * HSH], fp32, tag="gat")
                    gi = nc.gpsimd.indirect_dma_start(
                        out=gat[:], out_offset=None, in_=d_wih[:],
                        in_offset=bass.IndirectOffsetOnAxis(ap=idx_sb[:, 0:1], axis=0))
                    dep(gi.ins, last_pool.ins, reason="ring order")
                    last_pool = gi
                    nc.vector.tensor_tensor(out=gacc[:], in0=gacc[:], in1=gat[:],
                                            op=OP.add)

                # ======== HAM-warm dummy matmuls (keep PE clock at 2.4GHz) ====
                for _ in range(3):
                    nc.tensor.matmul(out=junk_ps[0:2, :], lhsT=mvg[bb][:],
                                     rhs=wfc[:, 0:512], start=True, stop=True,
                                     skip_group_check=True)

    nc.compile()
    return nc


_PROG_CACHE = {}


def _get_prog(T):
    if T not in _PROG_CACHE:
        _PROG_CACHE[T] = build_program(T)
    return _PROG_CACHE[T]


def prep_inputs(h0, W_ih, W_hh, b_ih, b_hh, W_fc, b_fc):
    """Host-side prep of per-core input arrays."""
    h0 = np.asarray(h0, np.float32)
    W_ih = np.asarray(W_ih, np.float32)
    W_hh = np.asarray(W_hh, np.float32)
    b_ih = np.asarray(b_ih, np.float32)
    b_hh = np.asarray(b_hh, np.float32)
    W_fc = np.asarray(W_fc, np.float32)
    b_fc = np.asarray(b_fc, np.float32)

    hh = (h0.astype(np.float64) @ W_hh.T.astype(np.float64)
          + b_hh.astype(np.float64) + b_ih.astype(np.float64)).astype(np.float32)

    gate_order = (0, 1, 3, 2)  # i, f, o, g  (torch order in rows is i,f,g,o)
    WihT = W_ih.T  # (V, 4H)
    ident = np.eye(B, dtype=np.float32)
    perm = np.zeros((128, B), np.float32)
    for j in range(B):
        perm[B + j, j] = 1.0

    in_maps = []
    for k in range(NC):
        wih_k = np.concatenate(
            [WihT[:, g * H + k * HSH: g * H + (k + 1) * HSH] for g in gate_order],
            axis=1)
        hh_k = np.concatenate(
            [hh[:, g * H + k * HSH: g * H + (k + 1) * HSH] for g in gate_order],
            axis=1)
        wfc_k = np.empty((NC, HSH, VSH), np.float32)
        for s in range(NC):
            q = k ^ s
            wfc_k[s] = 0.5 * W_fc[k * VSH:(k + 1) * VSH,
                                  q * HSH:(q + 1) * HSH].T
        cbv = np.empty((128, 1), np.float32)
        cbv[0:64, 0] = 8191.0 - k * VSH
        cbv[64:128, 0] = 8191.0 - k * VSH - 512.0
        in_maps.append({
            "wih": np.ascontiguousarray(wih_k),
            "wfc": wfc_k,
            "bias": b_fc[k * VSH:(k + 1) * VSH][None, :].copy(),
            "hh": np.ascontiguousarray(hh_k),
            "cb": cbv,
            "ident": ident,
            "perm": perm,
        })
    return in_maps


def kernel(h0, W_ih, W_hh, b_ih, b_hh, W_fc, b_fc, max_length):
    from concourse import bass_utils

    T = int(max_length)
    nc = _get_prog(T)
    in_maps = prep_inputs(h0, W_ih, W_hh, b_ih, b_hh, W_fc, b_fc)
    res = bass_utils.run_bass_kernel_spmd(nc, in_maps, core_ids=list(range(NC)))
    out = np.empty((B, V, T), np.float32)
    for k in range(NC):
        # per-core slab (T, B, VSH) -> out[:, k*VSH:(k+1)*VSH, :]
        out[:, k * VSH:(k + 1) * VSH, :] = res.results[k]["out"].transpose(1, 2, 0)
    return out



# revision 17
# speedup vs baseline: 1.2798x; 1.2798x over previous
"""Trainium2 Bass kernel for nn_DecoderRNN (LSTM decoder with argmax feedback).

V-sharded tensor-parallel over 8 cores (same math as baseline), with the
per-step collectives restructured for latency:
- Broadcast descriptors are PRE-GENERATED one step ahead on SWDGE queues
  1-3 (h: 3/2/2 frames, vg: 3/2/2 frames behind them); exact-count
  trigger_dma fires them the moment the data is ready. Desc generation
  (~0.85us x 14) moves off the critical path.
- Per-delta remote semaphores let the out-matmul consume h chunks as they
  arrive instead of waiting for all 7.
- Bias is added by DVE during PSUM->SBUF eviction (replaces two N=512
  fp32 bias matmuls on the PE).
- The W_ih row gather accumulates into gacc during DMA (compute_op=add).
- argmax/candidate-exchange/combine skipped at the last step.
"""
import numpy as np

V, H, B, NC = 8192, 1024, 64, 8
HSH = H // NC    # 128
VSH = V // NC    # 1024

PHYS = [0, 1, 2, 3, 6, 7, 4, 5]


def _dphys(d):
    # logical->physical NC map on trn2 is [0,1,2,3,6,7,4,5]; XOR-deltas map to
    return d ^ ((d & 4) >> 1)


QOF = [1, 2, 3, 1, 2, 3, 1]          # queue for delta d = QOF[d-1]
QCNT = {1: 3, 2: 2, 3: 2}            # frames per queue per exchange


def build_program(T):
    import concourse.mybir as mybir
    from concourse import bass, bacc, tile
    from concourse.bass import _add_dep_helper as dep

    fp32 = mybir.dt.float32
    AF = mybir.ActivationFunctionType
    OP = mybir.AluOpType

    nc = bacc.Bacc("TRN2", target_bir_lowering=False, debug=False,
                   num_devices=NC, num_swdge_queues=4)

    # ---- I/O ----
    d_wih = nc.dram_tensor("wih", [V, 4 * HSH], fp32, kind="ExternalInput").ap()
    d_wfc = nc.dram_tensor("wfc", [NC, HSH, VSH], fp32, kind="ExternalInput").ap()
    d_bias = nc.dram_tensor("bias", [128, 512], fp32, kind="ExternalInput").ap()
    d_hh = nc.dram_tensor("hh", [B, 4 * HSH], fp32, kind="ExternalInput").ap()
    d_cb = nc.dram_tensor("cb", [128, 1], fp32, kind="ExternalInput").ap()
    d_id = nc.dram_tensor("ident", [B, B], fp32, kind="ExternalInput").ap()
    d_pm = nc.dram_tensor("perm", [128, B], fp32, kind="ExternalInput").ap()
    d_out = nc.dram_tensor("out", [T, B, VSH], fp32, kind="ExternalOutput").ap()

    with tile.TileContext(nc) as tc:
        # ---- persistent SBUF ----
        wfc = nc.alloc_sbuf_tensor("wfc_sb", [128, NC * VSH], fp32).ap()
        bias = nc.alloc_sbuf_tensor("bias_sb", [128, 512], fp32).ap()
        gacc = nc.alloc_sbuf_tensor("gacc_sb", [B, 4 * HSH], fp32).ap()
        cC = nc.alloc_sbuf_tensor("c_sb", [B, HSH], fp32).ap()
        hts = [nc.alloc_sbuf_tensor(f"hts{i}", [128, B], fp32).ap() for i in range(2)]
        htb = [nc.alloc_sbuf_tensor(f"htb{i}", [128, 7 * B], fp32).ap() for i in range(2)]
        mvg = [nc.alloc_sbuf_tensor(f"mvg{i}", [128, 2], fp32).ap() for i in range(2)]
        rvg = [nc.alloc_sbuf_tensor(f"rvg{i}", [128, 16], fp32).ap() for i in range(2)]
        cb = nc.alloc_sbuf_tensor("cb_sb", [128, 1], fp32).ap()
        idn = nc.alloc_sbuf_tensor("idn_sb", [B, B], fp32).ap()
        perm = nc.alloc_sbuf_tensor("perm_sb", [128, B], fp32).ap()
        idx_sb = nc.alloc_sbuf_tensor("idx_sb", [B, 1], mybir.dt.int32).ap()
        pmin = nc.alloc_sbuf_tensor("pmin_sb", [128, 2], fp32).ap()

        # remote sems: per-delta for h (pipelined matmul), single for vg
        a_hd = [nc.alloc_semaphore(f"a_hd{d}") for d in range(1, 8)]
        a_vg = nc.alloc_semaphore("a_vg")
        # local send-complete sems (not waited on; required by the API)
        s_ht = nc.alloc_semaphore("s_ht")
        s_vg = nc.alloc_semaphore("s_vg")

        # ---- init ----
        for s in range(NC):
            nc.sync.dma_start(wfc[:, s * VSH:(s + 1) * VSH], d_wfc[s])
        nc.sync.dma_start(bias[:], d_bias[:])
        nc.sync.dma_start(gacc[:], d_hh[:])
        nc.sync.dma_start(cb[:], d_cb[:])
        nc.sync.dma_start(idn[:], d_id[:])
        nc.sync.dma_start(perm[:], d_pm[:])
        nc.vector.memset(cC[:], 0.0)

        def gen_h(t):
            bb = t % 2
            for d in range(1, 8):
                rd = [None] * 8
                rd[d] = (0, _dphys(d))
                nc.gpsimd.remote_dma_broadcast(
                    out_ap=htb[bb][:, (d - 1) * B:d * B], in_ap=hts[bb][:],
                    remote_sem=a_hd[d - 1], local_sem=s_ht, rdests=rd,
                    queue_num=QOF[d - 1])

        def gen_vg(t):
            bb = t % 2
            for d in range(1, 8):
                rd = [None] * 8
                rd[d] = (0, _dphys(d))
                nc.gpsimd.remote_dma_broadcast(
                    out_ap=rvg[bb][:, 2 * d:2 * d + 2], in_ap=mvg[bb][:],
                    remote_sem=a_vg, local_sem=s_vg, rdests=rd,
                    queue_num=QOF[d - 1])

        with tc.tile_pool(name="sb", bufs=3) as sb, \
             tc.tile_pool(name="sb2", bufs=2) as sb2, \
             tc.tile_pool(name="ps_out", bufs=2, space="PSUM") as ps_out, \
             tc.tile_pool(name="ps_tr", bufs=2, space="PSUM") as ps_tr, \
             tc.tile_pool(name="ps_pm", bufs=2, space="PSUM") as ps_pm, \
             tc.tile_pool(name="ps_junk", bufs=1, space="PSUM") as ps_junk:

            junk_ps = ps_junk.tile([128, 512], fp32, tag="junk")

            # pre-generate step-0 h frames (data-read deferred to trigger)
            gen_h(0)

            for t in range(T):
                bb = t % 2

                # ======== LSTM cell (gates -> c, h2=2h) ========
                tio = sb2.tile([B, 3 * HSH], fp32, tag="tio")
                tg_ = sb2.tile([B, HSH], fp32, tag="tg")
                nc.scalar.activation(tio[:], gacc[:, 0:3 * HSH], AF.Tanh, scale=0.5)
                nc.scalar.activation(tg_[:], gacc[:, 3 * HSH:4 * HSH], AF.Tanh)
                ti = tio[:, 0:HSH]
                tf = tio[:, HSH:2 * HSH]
                to = tio[:, 2 * HSH:3 * HSH]
                m1 = sb2.tile([B, HSH], fp32, tag="m1")
                s1 = sb2.tile([B, HSH], fp32, tag="s1")
                m2 = sb2.tile([B, HSH], fp32, tag="m2")
                s2 = sb2.tile([B, HSH], fp32, tag="s2")
                ss = sb2.tile([B, HSH], fp32, tag="ss")
                nc.vector.tensor_tensor(out=m1[:], in0=tf, in1=cC[:], op=OP.mult)
                nc.vector.tensor_tensor(out=s1[:], in0=cC[:], in1=m1[:], op=OP.add)
                nc.vector.tensor_tensor(out=m2[:], in0=ti, in1=tg_[:], op=OP.mult)
                nc.vector.tensor_tensor(out=s2[:], in0=tg_[:], in1=m2[:], op=OP.add)
                nc.vector.tensor_tensor(out=ss[:], in0=s1[:], in1=s2[:], op=OP.add)
                if t < T - 1:
                    nc.vector.tensor_scalar(out=cC[:], in0=ss[:], scalar1=0.5,
                                            scalar2=None, op0=OP.mult)
                tc2 = sb2.tile([B, HSH], fp32, tag="tc2")
                nc.scalar.activation(tc2[:], ss[:], AF.Tanh, scale=0.5)
                m3 = sb2.tile([B, HSH], fp32, tag="m3")
                h2 = sb2.tile([B, HSH], fp32, tag="h2")
                nc.vector.tensor_tensor(out=m3[:], in0=to, in1=tc2[:], op=OP.mult)
                nc.vector.tensor_tensor(out=h2[:], in0=tc2[:], in1=m3[:],
                                        op=OP.add)

                # ======== transpose h2 -> hts[bb] ========
                # step t-2's sends from hts[bb] have drained by induction:
                # queue FIFO + peers' step t-1 data already consumed.
                trp = ps_tr.tile([HSH, B], fp32, tag="trp")
                nc.tensor.transpose(out=trp[:], in_=h2[:], identity=idn[:])
                hci = nc.vector.tensor_copy(out=hts[bb][:], in_=trp[:])

                # ======== fire h exchange (descs pre-generated); then pre-gen
                # this step's vg frames behind them in the queue FIFO ========
                for q in (1, 2, 3):
                    trg = nc.gpsimd.trigger_dma(count=None, queue_num=q)
                    dep(trg.ins, hci.ins, reason="h data ready")
                gen_vg(t)

                # ======== out matmul: own chunk, then arrivals ========
                outp = ps_out.tile([128, 512], fp32, tag="outp")
                with tc.tile_critical():
                    for s in range(NC):
                        if s >= 1:
                            nc.tensor.wait_ge(a_hd[s - 1], 2 * (t + 1))
                        lhs = hts[bb][:] if s == 0 else htb[bb][:, (s - 1) * B:s * B]
                        nc.tensor.matmul(out=outp[0:64, :], lhsT=lhs,
                                         rhs=wfc[:, s * VSH:s * VSH + 512],
                                         start=(s == 0), stop=(s == NC - 1),
                                         tile_position=(0, 0),
                                         skip_group_check=True)
                        nc.tensor.matmul(out=outp[64:128, :], lhsT=lhs,
                                         rhs=wfc[:, s * VSH + 512:(s + 1) * VSH],
                                         start=(s == 0), stop=(s == NC - 1),
                                         tile_position=(0, 64),
                                         skip_group_check=True)

                # ======== bias add during PSUM->SBUF eviction ========
                osb = sb.tile([128, 512], fp32, tag="osb")
                nc.vector.tensor_tensor(out=osb[:], in0=outp[:], in1=bias[:],
                                        op=OP.add)
                nc.sync.dma_start(d_out[t, :, 0:512], osb[0:64, :])
                nc.sync.dma_start(d_out[t, :, 512:1024], osb[64:128, :])

                # ======== local argmax (post-bias values) ========
                mx = sb2.tile([128, 8], fp32, tag="mx")
                mi = sb2.tile([128, 8], mybir.dt.uint32, tag="mi")
                nc.vector.max(out=mx[:], in_=osb[:])
                nc.vector.max_index(out=mi[:], in_max=mx[:], in_values=osb[:])
                nc.vector.tensor_copy(out=mvg[bb][:, 0:1], in_=mx[:, 0:1])
                # code = cb - idx   (cb = 8191 - core*1024 - half*512)
                tsi = nc.vector.tensor_scalar(out=mvg[bb][:, 1:2], in0=mi[:, 0:1],
                                              scalar1=-1.0, scalar2=cb[:],
                                              op0=OP.mult, op1=OP.add)

                # ======== fire vg exchange; pre-gen next step's h frames ====
                for q in (1, 2, 3):
                    trg = nc.gpsimd.trigger_dma(count=None, queue_num=q)
                    dep(trg.ins, tsi.ins, reason="vg data ready")
                if t + 1 < T:
                    gen_h(t + 1)

                # ======== global argmax combine ========
                nc.vector.tensor_copy(out=rvg[bb][:, 0:2], in_=mvg[bb][:])
                vals = rvg[bb][:, 0:16:2]
                codes = rvg[bb][:, 1:16:2]
                with tc.tile_critical():
                    nc.vector.wait_ge(a_vg, 14 * (t + 1))
                    nc.vector.tensor_reduce(out=pmin[:, 0:1], in_=vals,
                                            axis=mybir.AxisListType.X, op=OP.max)
                msk = sb2.tile([128, 8], fp32, tag="msk")
                cm = sb2.tile([128, 8], fp32, tag="cm")
                nc.vector.tensor_scalar(out=msk[:], in0=vals,
                                        scalar1=pmin[:, 0:1], scalar2=None,
                                        op0=OP.is_equal)
                nc.vector.tensor_tensor(out=cm[:], in0=msk[:], in1=codes,
                                        op=OP.mult)
                nc.vector.tensor_reduce(out=pmin[:, 1:2], in_=cm[:],
                                        axis=mybir.AxisListType.X, op=OP.max)
                pmp = ps_pm.tile([B, 2], fp32, tag="pmp")
                nc.tensor.matmul(out=pmp[:], lhsT=perm[:], rhs=pmin[:],
                                 start=True, stop=True)
                geh = sb2.tile([B, 1], fp32, tag="geh")
                gel = sb2.tile([B, 1], fp32, tag="gel")
                ch = sb2.tile([B, 1], fp32, tag="ch")
                cl = sb2.tile([B, 1], fp32, tag="cl")
                cw = sb2.tile([B, 1], fp32, tag="cw")
                nc.vector.tensor_tensor(out=geh[:], in0=pmp[:, 0:1],
                                        in1=pmin[0:64, 0:1], op=OP.is_ge)
                nc.vector.tensor_tensor(out=gel[:], in0=pmin[0:64, 0:1],
                                        in1=pmp[:, 0:1], op=OP.is_ge)
                nc.vector.tensor_tensor(out=ch[:], in0=pmp[:, 1:2], in1=geh[:],
                                        op=OP.mult)
                nc.vector.tensor_tensor(out=cl[:], in0=pmin[0:64, 1:2], in1=gel[:],
                                        op=OP.mult)
                nc.vector.tensor_tensor(out=cw[:], in0=ch[:], in1=cl[:], op=OP.max)
                nc.vector.tensor_scalar(out=idx_sb[:], in0=cw[:], scalar1=-1.0,
                                        scalar2=8191.0, op0=OP.mult, op1=OP.add)

                # ======== gather W_ih.T rows, accumulate into gacc ========
                if t < T - 1:
                    nc.gpsimd.indirect_dma_start(
                        out=gacc[:], out_offset=None, in_=d_wih[:],
                        in_offset=bass.IndirectOffsetOnAxis(ap=idx_sb[:, 0:1],
                                                            axis=0),
                        compute_op=OP.add)

                # ======== PE keep-warm during exchange stalls ========
                for _ in range(3):
                    nc.tensor.matmul(out=junk_ps[0:2, 0:256], lhsT=mvg[bb][:],
                                     rhs=wfc[:, 0:256], start=True, stop=True,
                                     skip_group_check=True)

    nc.compile()
    return nc


_PROG_CACHE = {}


def _get_prog(T):
    if T not in _PROG_CACHE:
        _PROG_CACHE[T] = build_program(T)
    return _PROG_CACHE[T]


def prep_inputs(h0, W_ih, W_hh, b_ih, b_hh, W_fc, b_fc):
    """Host-side prep of per-core input arrays."""
    h0 = np.asarray(h0, np.float32)
    W_ih = np.asarray(W_ih, np.float32)
    W_hh = np.asarray(W_hh, np.float32)
    b_ih = np.asarray(b_ih, np.float32)
    b_hh = np.asarray(b_hh, np.float32)
    W_fc = np.asarray(W_fc, np.float32)
    b_fc = np.asarray(b_fc, np.float32)

    hh = (h0.astype(np.float64) @ W_hh.T.astype(np.float64)
          + b_hh.astype(np.float64) + b_ih.astype(np.float64)).astype(np.float32)

    gate_order = (0, 1, 3, 2)  # i, f, o, g  (torch order in rows is i,f,g,o)
    WihT = W_ih.T  # (V, 4H)
    ident = np.eye(B, dtype=np.float32)
    perm = np.zeros((128, B), np.float32)
    for j in range(B):
        perm[B + j, j] = 1.0

    in_maps = []
    for k in range(NC):
        wih_k = np.concatenate(
            [WihT[:, g * H + k * HSH: g * H + (k + 1) * HSH] for g in gate_order],
            axis=1)
        hh_k = np.concatenate(
            [hh[:, g * H + k * HSH: g * H + (k + 1) * HSH] for g in gate_order],
            axis=1)
        wfc_k = np.empty((NC, HSH, VSH), np.float32)
        for s in range(NC):
            q = k ^ s
            wfc_k[s] = 0.5 * W_fc[k * VSH:(k + 1) * VSH,
                                  q * HSH:(q + 1) * HSH].T
        cbv = np.empty((128, 1), np.float32)
        cbv[0:64, 0] = 8191.0 - k * VSH
        cbv[64:128, 0] = 8191.0 - k * VSH - 512.0
        bias_k = np.empty((128, 512), np.float32)
        bias_k[0:64, :] = b_fc[k * VSH:k * VSH + 512][None, :]
        bias_k[64:128, :] = b_fc[k * VSH + 512:(k + 1) * VSH][None, :]
        in_maps.append({
            "wih": np.ascontiguousarray(wih_k),
            "wfc": wfc_k,
            "bias": bias_k,
            "hh": np.ascontiguousarray(hh_k),
            "cb": cbv,
            "ident": ident,
            "perm": perm,
        })
    return in_maps


def kernel(h0, W_ih, W_hh, b_ih, b_hh, W_fc, b_fc, max_length):
    from concourse import bass_utils

    T = int(max_length)
    nc = _get_prog(T)
    in_maps = prep_inputs(h0, W_ih, W_hh, b_ih, b_hh, W_fc, b_fc)
    res = bass_utils.run_bass_kernel_spmd(nc, in_maps, core_ids=list(range(NC)))
    out = np.empty((B, V, T), np.float32)
    for k in range(NC):
        # per-core slab (T, B, VSH) -> out[:, k*VSH:(k+1)*VSH, :]
        out[:, k * VSH:(k + 1) * VSH, :] = \
            np.asarray(res.results[k]["out"]).reshape(T, B, VSH).transpose(1, 2, 0)
    return out
